# revision 54
# baseline (speedup 1.0000x reference)
"""Trainium2 Bass kernel for EvalMemoryReader (retrieval_knn).

Distributed plan (8 NeuronCores):
  A. memory-sharded argmax: fp32r matmul (own 1792 memory rows x all 1792
     queries) -> per-row argmax via DVE max8+find-index -> gaussian center
     (ym, xm) and alpha per memory row.  One AllGather ships the three
     gaussian rows for all 14336 memory rows to every core.
  B. query-sharded selection: each core computes scores s(m, q) for its own
     224 queries over ALL 14336 memory rows (fp32r matmul, 67 channels
     folding the gaussian), takes segment-16 maxima via a Pool max
     tournament, rank-51 of the 896 segmaxes via 7x(max8+match_replace)
     -> threshold t, then an in-place 4-pass sweep over the score row
     (v = t-s, z = min(v, -BIG*v), max8 -> 8 smallest survivors, count)
     -> exact v50/v51 midpoint tau per query.  AllGather tau (tiny).
  C. memory-sharded weights: fp32r matmul with tau folded in as a channel
     (psum = s - tau), premask z = min(ps*BIG, ps), exp -> bf16 weights;
     bf16 readout matmul in two k-halves + a norm row; one ReduceScatter
     with 65-row interleaving delivers summed values + norm; host divides.

kernel() takes FULL inputs, shards host-side, runs SPMD on cores 0-7.
"""

import math
import os

import ml_dtypes
import numpy as np

import concourse.bass as bass
import concourse.bacc as bacc
import concourse.mybir as mybir
from concourse.tile import TileContext

ND = 8
CK, CV, T, H, W = 64, 512, 8, 32, 56
HW = H * W              # 1792 queries
THW = T * HW            # 14336 memory locations
ML = THW // ND          # 1792 memory rows per core
NCH = HW // 128         # 14 chunks of 128
NB = 4                  # 448-wide free-dim chunks per 1792
NW = HW // NB           # 448
SEG = 16
NSEG = THW // SEG       # 896 segments per query (global)
NQ = HW // ND           # 224 queries per core
NJ = THW // NW          # 32 column blocks of 448 in the selection matmul
GD = 2.0 * 5.6 * 5.6    # 62.72
CG = math.sqrt(2.0 / GD)
BIG = 1.0e30
NEG = -1.0e30
MCV = CV // 128         # 4 output chunks

F32 = mybir.dt.float32
_R = mybir.dt.float32r
_F = mybir.dt.float32
_RMODE = __import__("os").environ.get("F32RMODE", "all")
F32R_AC = _R if _RMODE in ("all", "ac") else _F   # msb/q2c (phases A+C)
F32R_B = _R if _RMODE in ("all", "b") else _F     # mba/q2o (phase B')
F32R = F32R_AC
BF16 = mybir.dt.bfloat16
U32 = mybir.dt.uint32
ALU = mybir.AluOpType
ACT = mybir.ActivationFunctionType
AX = mybir.AxisListType


class _Trunc(Exception):
    pass


def _build():
    nc = bacc.Bacc(num_devices=ND)

    # msb rows: 0-63 own mk/4, 64-66 gaussian rows (runtime), 67 = -1 (tau
    # channel for phase C), 68 = -1 (|q|^2 channel for phase A)
    msb_d = nc.dram_tensor("msb", [69, ML], F32R, kind="ExternalInput")
    # q2c rows: 0-63 qk, 64 yv*cg, 65 xv*cg, 66 = -1, 67 = tau (runtime),
    # 68 = |q|^2/8
    q2c_d = nc.dram_tensor("q2c", [69, HW], F32R, kind="ExternalInput")
    # selection operands: full-memory channels + own-query columns
    mba_d = nc.dram_tensor("mba", [67, THW], F32R_B, kind="ExternalInput")
    q2o_d = nc.dram_tensor("q2o", [67, NQ], F32R_B, kind="ExternalInput")
    a8t_d = nc.dram_tensor("a8t", [128, NCH], F32, kind="ExternalInput")
    mvt_d = nc.dram_tensor("mvt", [128, NCH * CV], BF16, kind="ExternalInput")
    out_d = nc.dram_tensor("out", [CV // ND + 1, HW], F32, kind="ExternalOutput")

    iota16_c = nc.inline_tensor(
        np.broadcast_to(np.arange(16, dtype=np.float32), (128, 16)).copy(),
        name="iota16")
    ones_128x1_c = nc.inline_tensor(
        np.ones((128, 1), np.float32).astype(ml_dtypes.bfloat16), name="o128x1")
    thr56_c = nc.inline_tensor(
        np.broadcast_to(np.arange(1, H, dtype=np.float32) * W, (128, H - 1))
        .copy(), name="thr56")
    b20_c = nc.inline_tensor(
        np.full((128, 1), 1.0e20, np.float32), name="b20")

    # collective bounce buffers
    gau_l = nc.dram_tensor("gau_l", [3, ML], F32R)
    gau_g = nc.dram_tensor("gau_g", [ND, 3, ML], F32R, addr_space="Shared")
    tau_l = nc.dram_tensor("tau_l", [NQ, 1], F32R)
    tau_g = nc.dram_tensor("tau_g", [HW, 1], F32R, addr_space="Shared")
    scr = [nc.dram_tensor(f"scr{i}", [HW], F32R) for i in range(3)]
    # readout rows interleaved in groups of 65 per core: rows 65d..65d+63 are
    # value rows 64d..64d+63, row 65d+64 is a copy of the local norm row, so a
    # single ReduceScatter delivers each core its value slice + global norm.
    ro_l = nc.dram_tensor("ro_l", [(CV // ND + 1) * ND, HW], F32)
    rs_l = nc.dram_tensor("rs_l", [CV // ND + 1, HW], F32)

    groups = [list(range(ND))]

    from contextlib import ExitStack
    with TileContext(nc) as tc, ExitStack() as es:
        try:
            POOL_E = mybir.EngineType.Pool
            cpool = es.enter_context(tc.tile_pool(name="consts", bufs=1))
            def cload(ap, name):
                return cpool.tile_from(ap, force_copy=True, name=name,
                                       forced_dma_engine=POOL_E)
            msb = cload(msb_d[:], "msb_t")
            q2c = cload(q2c_d[:], "q2c_t")
            mba = cload(mba_d[:], "mba_t")
            q2o = cload(q2o_d[:], "q2o_t")
            a8t = cload(a8t_d[:], "a8t_t")
            iota16 = cload(iota16_c[:], "iota16_t")
            ones_cb = cload(ones_128x1_c[:], "ones_cb_t")
            thr56 = cload(thr56_c[:], "thr56_t")
            b20 = cload(b20_c[:], "b20_t")

            spool = es.enter_context(tc.tile_pool(name="smalls", bufs=1))
            ycg_t = spool.tile([128, NCH], F32R)
            xcg_t = spool.tile([128, NCH], F32R)
            alp_t = spool.tile([128, NCH], F32R)

            def part_to_row(scratch, row_ap, tile_ap):
                nc.sync.dma_start(
                    out=scratch[:].rearrange("(m q) -> q m", q=128), in_=tile_ap)
                nc.sync.dma_start(out=row_ap, in_=scratch[:])

            PH = int(os.environ.get("KPHASE", "99"))

            # ---------------- phase A: argmax per memory row ----------------
            with tc.tile_pool(name="psA", bufs=2, space="PSUM") as psA, \
                 tc.tile_pool(name="wkA", bufs=3) as wkA:
                for m in range(NCH):
                    ps = psA.tile([128, NB, 512], F32)
                    for j in range(NB):
                        nc.tensor.matmul(
                            ps[:, j, :NW],
                            lhsT=msb[0:69, m * 128:(m + 1) * 128],
                            rhs=q2c[0:69, j * NW:(j + 1) * NW],
                            start=True, stop=True)
                    u = wkA.tile([128, HW], F32, tag="u")
                    nc.scalar.activation(
                        u.rearrange("p (j n) -> p j n", n=NW), ps[:, :, :NW],
                        ACT.Copy)
                    m8 = wkA.tile([128, 8], F32, tag="m8")
                    i8 = wkA.tile([128, 8], U32, tag="i8")
                    nc.vector.max(m8, u)
                    nc.vector.max_index(i8, m8, u)
                    idxf = wkA.tile([128, 1], F32, tag="idxf")
                    nc.vector.tensor_copy(idxf, i8[:, 0:1])
                    xm = wkA.tile([128, 1], F32, tag="xm")
                    ym = wkA.tile([128, 1], F32, tag="ym")
                    jnk = wkA.tile([128, H - 1], F32, tag="jnk")
                    # y = #{k in 1..31 : k*W <= idx} = idx // W
                    nc.vector.tensor_scalar(jnk, thr56, idxf, None, op0=ALU.is_le,
                                            op1=ALU.add, accum_out=ym)
                    # x = idx - W*y
                    nc.vector.scalar_tensor_tensor(xm, ym, -float(W), idxf,
                                                   op0=ALU.mult, op1=ALU.add)
                    nc.scalar.activation(ycg_t[:, m:m + 1], ym, ACT.Copy,
                                         scale=CG)
                    nc.scalar.activation(xcg_t[:, m:m + 1], xm, ACT.Copy,
                                         scale=CG)
                    # alpha = a8 + (y^2 + x^2)/GD = a8 + ((y*cg)^2+(x*cg)^2)/2
                    ysq = wkA.tile([128, 1], F32, tag="ysq")
                    nc.vector.tensor_mul(ysq, ycg_t[:, m:m + 1].bitcast(F32), ycg_t[:, m:m + 1].bitcast(F32))
                    xsq = wkA.tile([128, 1], F32, tag="xsq")
                    nc.vector.tensor_mul(xsq, xcg_t[:, m:m + 1].bitcast(F32), xcg_t[:, m:m + 1].bitcast(F32))
                    ssum = wkA.tile([128, 1], F32, tag="ssum")
                    nc.vector.tensor_add(ssum, ysq, xsq)
                    hsum = wkA.tile([128, 1], F32, tag="hsum")
                    nc.scalar.activation(hsum, ssum, ACT.Copy, scale=0.5)
                    nc.vector.tensor_add(alp_t[:, m:m + 1], hsum, a8t[:, m:m + 1])
                    # stream this chunk's gaussian channels into msb rows
                    # 64..66 for phase C
                    for row, srct in ((64, ycg_t), (65, xcg_t), (66, alp_t)):
                        nc.sync.dma_start(
                            out=msb[row:row + 1, m * 128:(m + 1) * 128],
                            in_=srct[:, m:m + 1])

            # ship the gaussian rows for all memory rows to every core
            part_to_row(scr[0], gau_l[0:1, :], ycg_t[:])
            part_to_row(scr[1], gau_l[1:2, :], xcg_t[:])
            part_to_row(scr[2], gau_l[2:3, :], alp_t[:])
            if PH < 2:
                raise _Trunc()
            nc.gpsimd.collective_compute(
                "AllGather", ALU.bypass, replica_groups=groups,
                ins=[gau_l[:]], outs=[gau_g[:]])
            for r in range(3):
                nc.sync.dma_start(
                    out=mba[64 + r:65 + r, :].rearrange("a (d m) -> a d m",
                                                        d=ND),
                    in_=gau_g[:, r:r + 1, :].rearrange("d a m -> a d m"))

            if PH < 3:
                raise _Trunc()
            # -------- phase B: query-sharded selection over all memory ------
            with tc.tile_pool(name="sSpool", bufs=1) as sSpool, \
                 tc.tile_pool(name="wkB", bufs=1) as wkB, \
                 tc.tile_pool(name="wkT", bufs=2) as wkT, \
                 tc.tile_pool(name="psB", bufs=1, space="PSUM") as psB:
                sS = sSpool.tile([128, 2, THW], F32)
                seg = sSpool.tile([128, 2, NSEG], F32)
                for ci, (p0, pc) in enumerate(((0, 128), (128, 96))):
                    for j in range(NJ):
                        ps = psB.tile([128, 512], F32, tag=f"b{j % 4}")
                        nc.tensor.matmul(
                            ps[0:pc, :NW],
                            lhsT=q2o[:, p0:p0 + pc],
                            rhs=mba[:, j * NW:(j + 1) * NW],
                            start=True, stop=True)
                        nc.scalar.activation(
                            sS[0:pc, ci, j * NW:(j + 1) * NW], ps[0:pc, :NW],
                            ACT.Copy)
                        if j % 4 != 3:
                            continue
                        # segment-16 maxima for the group of 4 column blocks
                        # just copied (streams behind the copies); walrus only
                        # codegens vector ops on DVE, so segmented reduce it is
                        g = j // 4
                        gsl = slice(g * 4 * NW, (g + 1) * 4 * NW)
                        nc.vector.tensor_reduce(
                            seg[0:pc, ci, g * 112:(g + 1) * 112],
                            sS[0:pc, ci, gsl]
                            .rearrange("p (s k) -> p s k", k=SEG),
                            axis=AX.X, op=ALU.max)

                if PH < 4:
                    raise _Trunc()

                # ---- selection stages, explicitly interleaved so both
                # chunks' serial chains (rank51 -> v -> z -> max8 -> count)
                # overlap across DVE / Pool / ACT in-order queues ----
                CHK = ((0, 0, 128), (1, 128, 96))
                # DVE also carries rank51 + max8, so give it the smaller
                # share of the v/z sweeps and Pool the rest
                HCUT = (0, 7168, THW)
                t_cs, mn8s, css = {}, {}, {}

                def r51(ci, p0, pc):
                    ext = seg[0:pc, ci, :]
                    m8f = wkT.tile([128, 8], F32, tag=f"m8{ci}", name="m8")
                    m8 = m8f[0:pc]
                    for r in range(7):
                        nc.vector.max(m8, ext)
                        if r < 6:
                            nc.vector.match_replace(ext, m8, ext, NEG)
                    t_cf = wkT.tile([128, 1], F32, tag=f"t{ci}", name="tc")
                    t_cs[ci] = t_cf[0:pc]
                    nc.vector.tensor_copy(t_cs[ci], m8[:, 2:3])

                def vz(ci, p0, pc, h, eng):
                    # v = t - s ; z = min(v, -BIG*v), in place over the
                    # score part
                    sl = slice(HCUT[h], HCUT[h + 1])
                    Sh = sS[0:pc, ci, sl]
                    eng.tensor_scalar(Sh, Sh, t_cs[ci], -1.0,
                                      op0=ALU.subtract, op1=ALU.mult)
                    eng.scalar_tensor_tensor(
                        Sh, Sh, -BIG, Sh, op0=ALU.mult, op1=ALU.min)

                def m8h(ci, p0, pc, h):
                    # 8 smallest survivors of this half (as t-s, descending)
                    if ci not in mn8s:
                        cdf = wkT.tile([128, 16], F32, tag=f"cd{ci}", name="cd")
                        mn8s[ci] = cdf[0:pc]
                    sl = slice(HCUT[h], HCUT[h + 1])
                    nc.vector.max(mn8s[ci][:, h * 8:(h + 1) * 8],
                                  sS[0:pc, ci, sl])

                def cnt(ci, p0, pc, h):
                    # count survivors of this half on ACT:
                    # sign(z + 1e20) accumulate
                    sl = slice(HCUT[h], HCUT[h + 1])
                    Sh = sS[0:pc, ci, sl]
                    if ci not in css:
                        csf = wkT.tile([128, 2], F32, tag=f"cs{ci}", name="cs")
                        css[ci] = csf[0:pc]
                    nc.scalar.activation(Sh, Sh, ACT.Sign, bias=b20[0:pc],
                                         accum_out=css[ci][:, h:h + 1])

                def tau_fin(ci, p0, pc):
                    # tau = t - (asc8[e] + asc8[e-1])/2 with e = count - 50,
                    # count = (14336 + cs0 + cs1)/2
                    mn8 = wkT.tile([128, 8], F32, tag=f"mn{ci}", name="mn8")[0:pc]
                    nc.vector.max(mn8, mn8s[ci])
                    em05 = wkT.tile([128, 1], F32, tag=f"e{ci}", name="em")[0:pc]
                    nc.vector.tensor_reduce(em05, css[ci], axis=AX.X, op=ALU.add)
                    nc.vector.tensor_scalar(em05, em05, 0.5,
                                            float(THW) / 2 - 50.5,
                                            op0=ALU.mult, op1=ALU.add)
                    m1 = wkT.tile([128, 8], F32, tag=f"d8{ci}", name="m1")[0:pc]
                    nc.vector.tensor_scalar(m1, iota16[0:pc, 0:8], em05, 0.6,
                                            op0=ALU.subtract, op1=ALU.is_le)
                    m2 = wkT.tile([128, 8], F32, tag=f"d9{ci}", name="m2")[0:pc]
                    nc.vector.tensor_scalar(m2, iota16[0:pc, 0:8], em05, -0.6,
                                            op0=ALU.subtract, op1=ALU.is_ge)
                    mk2 = wkT.tile([128, 8], F32, tag=f"mk{ci}", name="mk2")[0:pc]
                    nc.vector.tensor_tensor(out=mk2, in0=m1, in1=m2,
                                            op=ALU.mult)
                    junk8 = wkT.tile([128, 8], F32, tag=f"j8{ci}", name="j8")[0:pc]
                    msum = wkT.tile([128, 1], F32, tag=f"ms{ci}", name="ms")[0:pc]
                    nc.vector.scalar_tensor_tensor(
                        junk8, mn8, 1.0, mk2, op0=ALU.mult, op1=ALU.mult,
                        accum_out=msum)
                    tau_c = wkT.tile([128, 1], F32R, tag=f"tv{ci}", name="tv")[0:pc]
                    nc.vector.scalar_tensor_tensor(
                        tau_c, msum, -0.5, t_cs[ci], op0=ALU.mult, op1=ALU.add)
                    nc.sync.dma_start(out=tau_l[p0:p0 + pc, :], in_=tau_c)

                r51(0, 0, 128)
                vz(0, 0, 128, 0, nc.vector)
                m8h(0, 0, 128, 0)
                cnt(0, 0, 128, 0)
                vz(0, 0, 128, 1, nc.vector)
                m8h(0, 0, 128, 1)
                cnt(0, 0, 128, 1)
                r51(1, 128, 96)
                vz(1, 128, 96, 0, nc.vector)
                m8h(1, 128, 96, 0)
                cnt(1, 128, 96, 0)
                vz(1, 128, 96, 1, nc.vector)
                m8h(1, 128, 96, 1)
                cnt(1, 128, 96, 1)
                tau_fin(0, 0, 128)
                tau_fin(1, 128, 96)

            if PH < 5:
                raise _Trunc()
            nc.gpsimd.collective_compute(
                "AllGather", ALU.bypass, replica_groups=groups,
                ins=[tau_l[:]], outs=[tau_g[:]])
            # tau (absolute) becomes q2c channel 67: psC = s - tau
            nc.sync.dma_start(out=q2c[67:68, :],
                              in_=tau_g[:].rearrange("q s -> s q"))

            if PH < 6:
                raise _Trunc()
            # ---------------- phase C: weights + readout ----------------
            with tc.tile_pool(name="Wpool", bufs=1) as Wpool, \
                 tc.tile_pool(name="mvp", bufs=1) as mvpool:
                Wt = Wpool.tile([128, NCH, ML], BF16)
                mvt = mvpool.tile_from(mvt_d[:], force_copy=True,
                                       forced_dma_engine=POOL_E)
                mvt3 = mvt.rearrange("p (k c) -> p k c", c=CV)
                with tc.tile_pool(name="psC", bufs=2, space="PSUM") as psC, \
                     tc.tile_pool(name="wkF", bufs=3) as wkF:
                    for k in range(NCH):
                        ps = psC.tile([128, NB, 512], F32)
                        for j in range(NB):
                            nc.tensor.matmul(
                                ps[:, j, :NW],
                                lhsT=msb[0:68, k * 128:(k + 1) * 128],
                                rhs=q2c[0:68, j * NW:(j + 1) * NW],
                                start=True, stop=True)
                        # psC = s - tau: copy to SBUF, then z = min(z*BIG, z)
                        # maps rejected entries (z<0) to -huge so exp(z) is
                        # the masked weight (Pool cannot touch PSUM and DVE
                        # may read PSUM only once per op, hence the copy).
                        z = wkF.tile([128, NB, NW], F32, tag="z")
                        nc.scalar.activation(z, ps[:, :, :NW], ACT.Copy)
                        nc.vector.scalar_tensor_tensor(
                            z, z, BIG, z, op0=ALU.mult, op1=ALU.min)
                        nc.scalar.activation(
                            Wt[:, k, :].rearrange("p (j n) -> p j n", n=NW),
                            z, ACT.Exp)

                if PH < 7:
                    raise _Trunc()
                # Readout in two k-halves with SBUF partial accumulation so
                # the first half's matmuls overlap phase C's tail.
                KH = NCH // 2
                with tc.tile_pool(name="wkO", bufs=2) as wkO, \
                     tc.tile_pool(name="accp", bufs=1) as accp:
                    pacc = accp.tile([128, MCV, NB, NW], F32)
                    with tc.tile_pool(name="psO", bufs=2, space="PSUM") as psO:
                        for half in range(2):
                            k0 = half * KH
                            for mc in range(MCV):
                                po = psO.tile([128, NB, 512], F32, tag="po")
                                for k in range(k0, k0 + KH):
                                    for j in range(NB):
                                        nc.tensor.matmul(
                                            po[:, j, :NW],
                                            lhsT=mvt3[:, k,
                                                      mc * 128:(mc + 1) * 128],
                                            rhs=Wt[:, k, j * NW:(j + 1) * NW],
                                            start=(k == k0),
                                            stop=(k == k0 + KH - 1))
                                if half == 0:
                                    nc.scalar.activation(
                                        pacc[:, mc], po[:, :, :NW], ACT.Copy)
                                else:
                                    ob = wkO.tile([128, NB, NW], F32, tag="ob")
                                    nc.vector.tensor_add(
                                        ob, pacc[:, mc], po[:, :, :NW])
                                    # value rows v=64d+i land at ro row 65d+i
                                    for h in range(2):
                                        d = 2 * mc + h
                                        nc.sync.dma_start(
                                            out=ro_l[d * 65:d * 65 + 64, :]
                                            .rearrange("r (j n) -> r j n",
                                                       n=NW),
                                            in_=ob[h * 64:(h + 1) * 64])
                    with tc.tile_pool(name="psN", bufs=1, space="PSUM") as psN:
                        pn = psN.tile([1, NB, 512], F32)
                        for k in range(NCH):
                            for j in range(NB):
                                nc.tensor.matmul(
                                    pn[:, j, :NW], lhsT=ones_cb,
                                    rhs=Wt[:, k, j * NW:(j + 1) * NW],
                                    start=(k == 0), stop=(k == NCH - 1))
                        nb_ = wkO.tile([1, NB, NW], F32, tag="nb")
                        nc.scalar.activation(nb_, pn[:, :, :NW], ACT.Copy)
                        for d in range(ND):
                            nc.sync.dma_start(
                                out=ro_l[d * 65 + 64:d * 65 + 65, :]
                                .rearrange("r (j n) -> r j n", n=NW),
                                in_=nb_)

            if PH < 8:
                raise _Trunc()
            # one ReduceScatter delivers 64 summed value rows + the summed
            # norm row to each core; the host does the division.
            nc.gpsimd.collective_compute(
                "ReduceScatter", ALU.add, replica_groups=groups,
                ins=[ro_l[:]], outs=[rs_l[:]])
            nc.sync.dma_start(out=out_d[:], in_=rs_l[:])

        except _Trunc:
            pass
    if not nc.is_finalized():
        nc.finalize()
    return nc


def _host_inputs(mk, qk, mv):
    mkf = np.asarray(mk, np.float32).reshape(CK, THW)
    qkf = np.asarray(qk, np.float32).reshape(CK, HW)
    mvf = np.asarray(mv, np.float32).reshape(CV, THW)
    c = (qkf * qkf).sum(0)
    a = (mkf * mkf).sum(0)
    yv = (np.arange(HW, dtype=np.float32) // W)
    xv = (np.arange(HW, dtype=np.float32) % W)

    q2c = np.zeros((69, HW), np.float32)
    q2c[0:64] = qkf
    q2c[64] = yv * CG
    q2c[65] = xv * CG
    q2c[66] = -1.0
    q2c[67] = 0.0
    q2c[68] = c / 8.0

    mba = np.zeros((67, THW), np.float32)
    mba[0:64] = mkf / 4.0

    in_maps = []
    for d in range(ND):
        sl = slice(d * ML, (d + 1) * ML)
        qsl = slice(d * NQ, (d + 1) * NQ)
        msb = np.zeros((69, ML), np.float32)
        msb[0:64] = mkf[:, sl] / 4.0
        msb[67] = -1.0
        msb[68] = -1.0
        q2o = np.zeros((67, NQ), np.float32)
        q2o[0:64] = qkf[:, qsl]
        q2o[64] = (yv * CG)[qsl]
        q2o[65] = (xv * CG)[qsl]
        q2o[66] = -1.0
        a8t = np.ascontiguousarray(
            (a[sl] / 8.0).reshape(NCH, 128).T.astype(np.float32))
        mvt = np.ascontiguousarray(
            mvf[:, sl].T.reshape(NCH, 128, CV).transpose(1, 0, 2)
            .reshape(128, NCH * CV)).astype(ml_dtypes.bfloat16)
        in_maps.append({
            "msb": msb, "q2c": q2c, "mba": mba, "q2o": q2o,
            "a8t": a8t, "mvt": mvt,
        })
    return in_maps


_NC_CACHE = {}


def _get_nc():
    if "nc" not in _NC_CACHE:
        _NC_CACHE["nc"] = _build()
    return _NC_CACHE["nc"]


def assemble(per_core_outs):
    """Each core returns [65, HW]: 64 summed value rows + the summed norm
    row. Normalize host-side and concatenate the 8 slices."""
    parts = []
    for o in per_core_outs:
        o = np.asarray(o, np.float32)
        parts.append(o[0:CV // ND] / o[CV // ND:CV // ND + 1])
    return np.concatenate(parts, axis=0).reshape(1, CV, H, W)


def kernel(mk, qk, mv):
    from concourse.bass_utils import run_bass_kernel_spmd
    in_maps = _host_inputs(mk, qk, mv)
    nc = _get_nc()
    res = run_bass_kernel_spmd(nc, in_maps, core_ids=list(range(ND)))
    return assemble([res.results[d]["out"] for d in range(ND)])
